# revision 20
# baseline (speedup 1.0000x reference)
"""DSA single-head attention block (dwconv QK + PEG V + attention + MLP) on 8 trn2 cores.

Sharding: data-parallel over batch (8 images -> 8 cores), weights replicated.
Self-contained: hardcodes shapes B=8, C=256, H=W=64, hidden=1024.

Per-core pipeline:
  - depthwise 3x3 convs (QK and PEG+residual) on the tensor engine via
    per-tap diagonal matrices accumulated in PSUM (bf16)
  - attention computed as logitsT[m,n] = k^T q tiles; exp on ACT straight out
    of PSUM (1/sqrt(C) folded into the activation scale); zT[n,c] accumulated
    with an extra ones column in vT producing the softmax denominators free
  - per-partition normalize, DMA-transpose back to [c,n]
  - MLP p1 -> BN -> p2 -> SiLU -> p3(*gamma) in bf16, f32 residual add
"""

import os
import sys

for _p in ("/opt/trn_rl_repo", os.path.expanduser("~/.axon_site/_ro/trn_rl_repo")):
    if os.path.isdir(_p) and _p not in sys.path:
        sys.path.insert(0, _p)

from contextlib import ExitStack

import ml_dtypes
import numpy as np

import concourse.bass as bass
import concourse.tile as tile
from concourse import bacc, mybir
from concourse.bass_utils import run_bass_kernel_spmd
from concourse.masks import make_identity
from concourse import dve_ops as _dvo
from concourse.dve_spec import Spec as _DveSpec, Src0, Src1, C0, C1, C2, One
from concourse.dve_spec import lower as _dve_lower
from concourse.dve_uop import DveOpSpec as _DveOpSpec

# exp(y/16) = (exp(y/32))^2 ~= p(y)^2 with p = degree-3 Taylor of exp(y/32);
# rel err < 2e-3 for |y| <= 11, far below the fp8e4 output rounding. The
# squared form needs only the 3 wired scalar slots (Src1 [P,1] broadcast
# crashes this silicon path).
EXP_C = (1.0 / 32, 1.0 / 2048, 1.0 / 196608)


def _register_exp_op():
    name = "EXP16SQ_ANT"
    for o in _dvo.OPS:
        if o.name == name:
            return o

    def _exp_ref(in0, in1, c0, c1, c2):
        x = in0.astype("float32")
        t = 1.0 + x * (c0 + x * (c1 + x * c2))
        return t * t

    _t = One + Src0 * (C0 + Src0 * (C1 + Src0 * C2))
    spec = _DveSpec(body=_t * _t, reference=_exp_ref)
    op = _dvo.DveOp(name, spec, subdim=False, uops_sha={})
    _dvo.OPS.append(op)
    row = _dvo._CUSTOM_DVE_ROW_BASE + len(_dvo.OPS) - 1
    assert row < 0x20
    _dvo._SUB_OPCODE_FOR_NAME[name] = row
    _dvo.CUSTOM_DVE_SPECS[name] = spec
    for ver in ("v3", "v4"):
        r = _DveOpSpec(name=name, opcode=row, uops=_dve_lower(spec, ver=ver),
                       rd1_en=False)
        op.uops_sha[ver] = r.sha(ver)
    return op


_EXP_OP = _register_exp_op()

F32 = mybir.dt.float32
BF16 = mybir.dt.bfloat16
FP8 = mybir.dt.float8e4
AF = mybir.ActivationFunctionType
ALU = mybir.AluOpType

P = 128          # partitions
C = 256          # channels
H = W = 64
N = H * W        # 4096 tokens
HP = H + 2       # padded spatial
PADN = HP * HP   # 4356
NT = 512         # token block (q columns per attention outer step)
NB = N // NT     # 8
MT = 32          # number of 128-wide m tiles
HID = 1024
EPS = 1e-5
N_CORES = 8

_cache = {}


def _conv_pe(nc, pmm, diag_sb, base, xp3b, dst_evict):
    """Depthwise 3x3 conv on PE: 9 diag-matmuls accumulated in PSUM per
    512-token block; dst_evict(nb, psum_ap) consumes each block."""
    for nb in range(NB):
        vp = pmm.tile([P, NT], F32, name="convp", tag="mm")
        ti = 0
        for dy in range(3):
            for dx in range(3):
                win = xp3b[:, dy + nb * 8:dy + nb * 8 + 8, dx:dx + W]
                nc.tensor.matmul(
                    vp[:], diag_sb[:, (base + ti) * P:(base + ti + 1) * P],
                    win, start=(ti == 0), stop=(ti == 8))
                ti += 1
        dst_evict(nb, vp)


def _build_program(dbg=False, reps=1):
    nc = bacc.Bacc("TRN2", target_bir_lowering=False, debug=False,
                   num_devices=N_CORES)

    x_ap = nc.dram_tensor("x", [2, P, H, W], F32, kind="ExternalInput").ap()
    qkd_ap = nc.dram_tensor("qkd", [2, 9, P, P], BF16, kind="ExternalInput").ap()
    qks_ap = nc.dram_tensor("qks", [2, P, 1], F32, kind="ExternalInput").ap()
    qkt_ap = nc.dram_tensor("qkt", [2, P, 1], F32, kind="ExternalInput").ap()
    pegd_ap = nc.dram_tensor("pegd", [2, 9, P, P], BF16, kind="ExternalInput").ap()
    p1t_ap = nc.dram_tensor("p1t", [2, P, C], BF16, kind="ExternalInput").ap()
    pbs_ap = nc.dram_tensor("pbs", [2, P, 1], F32, kind="ExternalInput").ap()
    pbt_ap = nc.dram_tensor("pbt", [2, P, 1], F32, kind="ExternalInput").ap()
    p2t_ap = nc.dram_tensor("p2t", [2, P, HID], BF16, kind="ExternalInput").ap()
    p3t_ap = nc.dram_tensor("p3t", [8, P, C], BF16, kind="ExternalInput").ap()
    out_ap = nc.dram_tensor("out", [2, P, N], F32, kind="ExternalOutput").ap()
    dbg_aps = {}
    if dbg:
        for nm in ("dq", "dk", "dv0", "dv1", "dz0", "dz1"):
            dbg_aps[nm] = nc.dram_tensor(nm, [P, N], BF16, kind="ExternalOutput").ap()
        dbg_aps["dvT"] = nc.dram_tensor("dvT", [P, MT * 257], BF16, kind="ExternalOutput").ap()

    with tile.TileContext(nc) as tc, ExitStack() as ctx:
        pers = ctx.enter_context(tc.tile_pool(name="pers", bufs=1))
        pmm = ctx.enter_context(tc.tile_pool(name="pmm", bufs=4, space="PSUM"))
        pzt = ctx.enter_context(tc.tile_pool(name="pzt", bufs=4, space="PSUM"))

        # ---- persistent SBUF tensors ----
        x_sb = [pers.tile([P, N], F32, name=f"x{ct}") for ct in range(2)]
        xpad_bf = [pers.tile([P, PADN], BF16, name=f"xpadbf{ct}") for ct in range(2)]
        qk_sb = [pers.tile([P, N], BF16, name=f"qk{ct}") for ct in range(2)]  # q, k
        v_sb = [pers.tile([P, N], BF16, name=f"v{ct}") for ct in range(2)]
        vT = pers.tile([P, MT * 257], FP8, name="vT")
        z_sb = [pers.tile([P, N], BF16, name=f"z{ct}") for ct in range(2)]

        qkd_sb = pers.tile([P, 18 * P], BF16, name="qkd")
        qks_sb = [pers.tile([P, 1], F32, name=f"qks{ct}") for ct in range(2)]
        qkt_sb = [pers.tile([P, 1], F32, name=f"qkt{ct}") for ct in range(2)]
        pegd_sb = pers.tile([P, 18 * P], BF16, name="pegd")
        p1t_sb = pers.tile([P, 2 * C], BF16, name="p1t")
        pbs_sb = [pers.tile([P, 1], F32, name=f"pbs{ct}") for ct in range(2)]
        pbt_sb = [pers.tile([P, 1], F32, name=f"pbt{ct}") for ct in range(2)]
        p2t_sb = pers.tile([P, 2 * HID], BF16, name="p2t")
        p3t_sb = pers.tile([P, 8 * C], BF16, name="p3t")

        xp3b = [t.rearrange("p (h w) -> p h w", h=HP, w=HP) for t in xpad_bf]
        vT3 = vT.rearrange("p (m c) -> p m c", m=MT, c=257)

        # ---- load inputs / weights ----
        for ct in range(2):
            xb3 = xp3b[ct]
            nc.gpsimd.memset(xb3[:, 0, :], 0.0)
            nc.gpsimd.memset(xb3[:, 65, :], 0.0)
            nc.gpsimd.memset(xb3[:, 1:65, 0], 0.0)
            nc.gpsimd.memset(xb3[:, 1:65, 65], 0.0)
            for half in range(2):
                nc.sync.dma_start(
                    x_sb[ct][:, half * (N // 2):(half + 1) * (N // 2)],
                    x_ap[ct].rearrange("p h w -> p (h w)")[:, half * (N // 2):
                                                          (half + 1) * (N // 2)])
            nc.sync.dma_start(qks_sb[ct][:], qks_ap[ct])
            nc.sync.dma_start(qkt_sb[ct][:], qkt_ap[ct])
            nc.sync.dma_start(pbs_sb[ct][:], pbs_ap[ct])
            nc.sync.dma_start(pbt_sb[ct][:], pbt_ap[ct])
            nc.sync.dma_start(p1t_sb[:, ct * C:(ct + 1) * C], p1t_ap[ct])
            nc.sync.dma_start(p2t_sb[:, ct * HID:(ct + 1) * HID], p2t_ap[ct])
            for t in range(9):
                nc.sync.dma_start(
                    pegd_sb[:, (ct * 9 + t) * P:(ct * 9 + t + 1) * P],
                    pegd_ap[ct, t])
                nc.sync.dma_start(
                    qkd_sb[:, (ct * 9 + t) * P:(ct * 9 + t + 1) * P],
                    qkd_ap[ct, t])
        for kt in range(8):
            nc.sync.dma_start(p3t_sb[:, kt * C:(kt + 1) * C], p3t_ap[kt])

        # constants / pools used by the compute body
        att_pool = ctx.enter_context(tc.tile_pool(name="att", bufs=3))
        epi_pool = ctx.enter_context(tc.tile_pool(name="epi", bufs=4))
        proj_pool = ctx.enter_context(tc.tile_pool(name="proj", bufs=4))
        out_pool = ctx.enter_context(tc.tile_pool(name="outp", bufs=4))
        ident = pers.tile([P, P], BF16, name="ident")
        make_identity(nc, ident)
        nc.gpsimd.memset(vT3[:, :, 256:257], 1.0)

        def emit_body():
            # bf16 padded input (conv rhs); borders stay zero from the memset
            for ct in range(2):
                for half in range(2):
                    nc.vector.tensor_copy(
                        xp3b[ct][:, 1 + half * 32:1 + (half + 1) * 32, 1:65],
                        x_sb[ct].rearrange("p (h w) -> p h w", h=H, w=W)
                        [:, half * 32:(half + 1) * 32, :])

            # ---- PEG conv (V branch) on PE ----
            for ct in range(2):
                def evict_v(nb, vp, ct=ct):
                    nc.scalar.copy(v_sb[ct][:, nb * NT:(nb + 1) * NT], vp[:])
                _conv_pe(nc, pmm, pegd_sb, ct * 9, xp3b[ct], evict_v)

            # ---- QK conv interleaved with vT transposes (PE works while DVE
            # evicts the transposed tiles) ----
            def emit_vt_group(g):
                # 8 transposes per group g in 0..7
                for i in range(8):
                    mi, ct = divmod(g * 8 + i, 2)
                    vtp = pzt.tile([P, P], BF16, name="vtp",
                                   tag=f"zt{(g * 8 + i) % 4}", bufs=1)
                    nc.tensor.transpose(
                        vtp[:], v_sb[ct][:, mi * P:(mi + 1) * P], ident[:])
                    nc.vector.tensor_copy(
                        vT3[:, mi, ct * P:(ct + 1) * P], vtp[:])

            g = 0
            for ct in range(2):
                def evict_qk(nb, vp, ct=ct):
                    nc.scalar.activation(
                        qk_sb[ct][:, nb * NT:(nb + 1) * NT], vp[:], AF.Silu,
                        bias=qkt_sb[ct][:], scale=qks_sb[ct][:])
                for nb in range(NB):
                    if nb % 2 == 0:
                        emit_vt_group(g)
                        g += 1
                    vp = pmm.tile([P, NT], F32, name="convp", tag="mm")
                    ti = 0
                    for dy in range(3):
                        for dx in range(3):
                            win = xp3b[ct][:, dy + nb * 8:dy + nb * 8 + 8,
                                           dx:dx + W]
                            nc.tensor.matmul(
                                vp[:], qkd_sb[:, (ct * 9 + ti) * P:
                                              (ct * 9 + ti + 1) * P],
                                win, start=(ti == 0), stop=(ti == 8))
                            ti += 1
                    evict_qk(nb, vp)

            # ---- attention (software-pipelined: lg of pair i+1 before zt of i) ----
            q, k = qk_sb[0], qk_sb[1]
            NPAIR = MT // 2

            def emit_lg_pair(nt, mp):
                tiles = []
                for h in range(2):
                    mi = 2 * mp + h
                    lg = pmm.tile([P, NT], F32, name="lg", tag="mm")
                    nc.tensor.matmul(
                        lg[:], k[:, mi * P:(mi + 1) * P],
                        q[:, nt * NT:(nt + 1) * NT], start=True, stop=True)
                    tiles.append(lg)
                return tiles

            def pair_seq():
                for nt in range(NB):
                    for mp in range(NPAIR):
                        yield nt, mp

            seq = list(pair_seq())
            pend = {}
            pend[seq[0]] = emit_lg_pair(*seq[0])
            pend[seq[1]] = emit_lg_pair(*seq[1])
            for idx, (nt, mp) in enumerate(seq):
                if mp == 0:
                    zt = [pzt.tile([P, 257], F32, name=f"ztp{j}", tag=f"zt{j}",
                                   bufs=1) for j in range(4)]
                lg2 = pend.pop((nt, mp))
                et = att_pool.tile([P, 2 * NT], FP8, name="et", tag="et", bufs=3)
                # two halves of the pair exp'd concurrently on ACT and DVE
                for h in range(2):
                    dve = (h + mp) % 2 == 0 and (2 * mp + h) % 10 < 8
                    if dve:
                        nc.vector._custom_dve(
                            _EXP_OP, out=et[:, h * NT:(h + 1) * NT],
                            in0=lg2[h][:],
                            s0=EXP_C[0], s1=EXP_C[1], imm2=EXP_C[2])
                    else:
                        nc.scalar.activation(et[:, h * NT:(h + 1) * NT],
                                             lg2[h][:], AF.Exp, scale=1.0 / 16.0)
                if idx + 2 < len(seq):
                    pend[seq[idx + 2]] = emit_lg_pair(*seq[idx + 2])
                et3 = et.rearrange("p (h n) -> p h n", h=2, n=NT)
                for j in range(4):
                    nc.tensor.matmul(
                        zt[j][:],
                        et3[:, :, j * P:(j + 1) * P],
                        vT3[:, 2 * mp:2 * mp + 2, :],
                        start=(mp == 0), stop=(mp == NPAIR - 1),
                        perf_mode=mybir.MatmulPerfMode.DoubleRow)
                if mp == NPAIR - 1:
                    for j in range(4):
                        recip = epi_pool.tile([P, 1], F32, name="recip",
                                              tag="recip")
                        nc.vector.reciprocal(recip[:], zt[j][:, 256:257])
                        zn = epi_pool.tile([P, C], BF16, name="zn", tag="zn")
                        nc.vector.tensor_scalar(zn[:], zt[j][:, :C], recip[:],
                                                None, ALU.mult)
                        for ct in range(2):
                            tp2 = pzt.tile([P, P], BF16, name="tp2",
                                           tag=f"zt{j}", bufs=1)
                            nc.tensor.transpose(tp2[:], zn[:, ct * P:(ct + 1) * P],
                                                ident[:])
                            nc.vector.tensor_copy(
                                z_sb[ct][:, nt * NT + j * P:
                                          nt * NT + (j + 1) * P], tp2[:])

            if dbg:
                nc.sync.dma_start(dbg_aps["dq"][:], qk_sb[0][:])
                nc.sync.dma_start(dbg_aps["dk"][:], qk_sb[1][:])
                nc.sync.dma_start(dbg_aps["dz0"][:], z_sb[0][:])
                nc.sync.dma_start(dbg_aps["dz1"][:], z_sb[1][:])
                nc.sync.dma_start(dbg_aps["dvT"][:], vT[:])

            # ---- projection MLP: p1 -> BN -> p2 -> SiLU -> p3(*gamma) + x ----
            for nt in range(NB):
                ns = slice(nt * NT, (nt + 1) * NT)
                h1 = []
                for ot in range(2):
                    h1p = pzt.tile([P, NT], F32, name="h1p", tag=f"zt{ot}", bufs=1)
                    for kt in range(2):
                        nc.tensor.matmul(
                            h1p[:],
                            p1t_sb[:, kt * C + ot * P:kt * C + (ot + 1) * P],
                            z_sb[kt][:, ns], start=(kt == 0), stop=(kt == 1))
                    h1t = proj_pool.tile([P, NT], BF16, name="h1t", tag="h1")
                    nc.vector.tensor_scalar(h1t[:], h1p[:], pbs_sb[ot][:],
                                            pbt_sb[ot][:], ALU.mult, ALU.add)
                    h1.append(h1t)
                h2 = []
                for ht in range(8):
                    h2p = pmm.tile([P, NT], F32, name="h2p", tag="mm")
                    for kt in range(2):
                        nc.tensor.matmul(
                            h2p[:],
                            p2t_sb[:, kt * HID + ht * P:kt * HID + (ht + 1) * P],
                            h1[kt][:], start=(kt == 0), stop=(kt == 1))
                    h2t = proj_pool.tile([P, NT], BF16, name="h2t", tag="h2",
                                         bufs=10)
                    nc.scalar.activation(h2t[:], h2p[:], AF.Silu)
                    h2.append(h2t)
                for ot in range(2):
                    zfp = pzt.tile([P, NT], F32, name="zfp", tag=f"zt{ot + 2}",
                                   bufs=1)
                    for kt in range(8):
                        nc.tensor.matmul(
                            zfp[:],
                            p3t_sb[:, kt * C + ot * P:kt * C + (ot + 1) * P],
                            h2[kt][:], start=(kt == 0), stop=(kt == 7))
                    ob = out_pool.tile([P, NT], F32, name="ob", tag="ob")
                    nc.vector.tensor_tensor(ob[:], zfp[:], x_sb[ot][:, ns], ALU.add)
                    nc.sync.dma_start(out_ap[ot][:, ns], ob[:])

        for _rep in range(reps):
            emit_body()

    nc.finalize()
    return nc


def _prep_inputs(x, qk_w, qk_g, qk_b, qk_m, qk_v, peg_w,
                 p1_w, pb_g, pb_b, pb_m, pb_v, p2_w, p3_w, gamma):
    f32 = np.float32
    bf16 = ml_dtypes.bfloat16

    def diag_taps(w9):
        d = np.zeros((2, 9, P, P), f32)
        idx = np.arange(P)
        for ct in range(2):
            for t in range(9):
                d[ct, t, idx, idx] = w9[ct * P:(ct + 1) * P, t]
        return d

    qks = (qk_g / np.sqrt(qk_v + EPS)).astype(f32)
    qkt = (qk_b - qk_m * qks).astype(f32)
    qkd = diag_taps(np.asarray(qk_w, f32).reshape(C, 9))

    pegw = np.asarray(peg_w, f32).reshape(C, 9).copy()
    pegw[:, 4] += 1.0  # fold +x residual into center tap
    pegd = diag_taps(pegw)

    pbs = (pb_g / np.sqrt(pb_v + EPS)).astype(f32)
    pbt = (pb_b - pb_m * pbs).astype(f32)

    p1t = np.ascontiguousarray(np.asarray(p1_w, f32).T).reshape(2, P, C)
    p2t = np.ascontiguousarray(np.asarray(p2_w, f32).T).reshape(2, P, HID)
    p3g = np.asarray(p3_w, f32) * np.asarray(gamma, f32)[:, None]
    p3t = np.ascontiguousarray(p3g.T).reshape(8, P, C)

    shared = {
        "qkd": qkd.astype(bf16),
        "qks": qks.reshape(2, P, 1).astype(f32),
        "qkt": qkt.reshape(2, P, 1).astype(f32),
        "pegd": pegd.astype(bf16),
        "p1t": p1t.astype(bf16),
        "pbs": pbs.reshape(2, P, 1).astype(f32),
        "pbt": pbt.reshape(2, P, 1).astype(f32),
        "p2t": p2t.astype(bf16),
        "p3t": p3t.astype(bf16),
    }
    xs = np.asarray(x, f32).reshape(8, 2, P, H, W)
    return [dict(shared, x=np.ascontiguousarray(xs[i])) for i in range(N_CORES)]


def kernel(**inputs):
    if "nc" not in _cache:
        _cache["nc"] = _build_program()
    nc = _cache["nc"]
    in_maps = _prep_inputs(**inputs)
    res = run_bass_kernel_spmd(nc, in_maps, list(range(N_CORES)))
    _cache["last_result"] = res
    out = np.stack([res.results[i]["out"].reshape(C, H, W)
                    for i in range(N_CORES)])
    return out.astype(np.float32)


# revision 21
# speedup vs baseline: 1.0635x; 1.0635x over previous
"""DSA single-head attention block (dwconv QK + PEG V + attention + MLP) on 8 trn2 cores.

Sharding: data-parallel over batch (8 images -> 8 cores), weights replicated.
Self-contained: hardcodes shapes B=8, C=256, H=W=64, hidden=1024.

Per-core pipeline:
  - depthwise 3x3 convs (QK and PEG+residual) on the tensor engine via
    per-tap diagonal matrices accumulated in PSUM (bf16)
  - attention computed as logitsT[m,n] = k^T q tiles; exp on ACT straight out
    of PSUM (1/sqrt(C) folded into the activation scale); zT[n,c] accumulated
    with an extra ones column in vT producing the softmax denominators free
  - per-partition normalize, DMA-transpose back to [c,n]
  - MLP p1 -> BN -> p2 -> SiLU -> p3(*gamma) in bf16, f32 residual add
"""

import os
import sys

for _p in ("/opt/trn_rl_repo", os.path.expanduser("~/.axon_site/_ro/trn_rl_repo")):
    if os.path.isdir(_p) and _p not in sys.path:
        sys.path.insert(0, _p)

from contextlib import ExitStack

import ml_dtypes
import numpy as np

import concourse.bass as bass
import concourse.tile as tile
from concourse import bacc, mybir
from concourse.bass_utils import run_bass_kernel_spmd
from concourse.masks import make_identity
from concourse import dve_ops as _dvo
from concourse.dve_spec import Spec as _DveSpec, Src0, Src1, C0, C1, C2, One
from concourse.dve_spec import lower as _dve_lower
from concourse.dve_uop import DveOpSpec as _DveOpSpec

# exp(y/16) = (exp(y/32))^2 ~= p(y)^2 with p = degree-3 Taylor of exp(y/32);
# rel err < 2e-3 for |y| <= 11, far below the fp8e4 output rounding. The
# squared form needs only the 3 wired scalar slots (Src1 [P,1] broadcast
# crashes this silicon path).
EXP_C = (1.0 / 32, 1.0 / 2048, 1.0 / 196608)


def _register_exp_op():
    name = "EXP16SQ_ANT"
    for o in _dvo.OPS:
        if o.name == name:
            return o

    def _exp_ref(in0, in1, c0, c1, c2):
        x = in0.astype("float32")
        t = 1.0 + x * (c0 + x * (c1 + x * c2))
        return t * t

    _t = One + Src0 * (C0 + Src0 * (C1 + Src0 * C2))
    spec = _DveSpec(body=_t * _t, reference=_exp_ref)
    op = _dvo.DveOp(name, spec, subdim=False, uops_sha={})
    _dvo.OPS.append(op)
    row = _dvo._CUSTOM_DVE_ROW_BASE + len(_dvo.OPS) - 1
    assert row < 0x20
    _dvo._SUB_OPCODE_FOR_NAME[name] = row
    _dvo.CUSTOM_DVE_SPECS[name] = spec
    for ver in ("v3", "v4"):
        r = _DveOpSpec(name=name, opcode=row, uops=_dve_lower(spec, ver=ver),
                       rd1_en=False)
        op.uops_sha[ver] = r.sha(ver)
    return op


_EXP_OP = _register_exp_op()

F32 = mybir.dt.float32
BF16 = mybir.dt.bfloat16
FP8 = mybir.dt.float8e4
AF = mybir.ActivationFunctionType
ALU = mybir.AluOpType

P = 128          # partitions
C = 256          # channels
H = W = 64
N = H * W        # 4096 tokens
HP = H + 2       # padded spatial
PADN = HP * HP   # 4356
NT = 512         # token block (q columns per attention outer step)
NB = N // NT     # 8
MT = 32          # number of 128-wide m tiles
HID = 1024
EPS = 1e-5
N_CORES = 8

_cache = {}


def _conv_pe(nc, pmm, diag_sb, base, xp3b, dst_evict):
    """Depthwise 3x3 conv on PE: 9 diag-matmuls accumulated in PSUM per
    512-token block; dst_evict(nb, psum_ap) consumes each block."""
    for nb in range(NB):
        vp = pmm.tile([P, NT], F32, name="convp", tag="mm")
        ti = 0
        for dy in range(3):
            for dx in range(3):
                win = xp3b[:, dy + nb * 8:dy + nb * 8 + 8, dx:dx + W]
                nc.tensor.matmul(
                    vp[:], diag_sb[:, (base + ti) * P:(base + ti + 1) * P],
                    win, start=(ti == 0), stop=(ti == 8))
                ti += 1
        dst_evict(nb, vp)


def _build_program(dbg=False, reps=1):
    nc = bacc.Bacc("TRN2", target_bir_lowering=False, debug=False,
                   num_devices=N_CORES)

    x_ap = nc.dram_tensor("x", [2, P, H, W], F32, kind="ExternalInput").ap()
    qkd_ap = nc.dram_tensor("qkd", [2, 9, P, P], BF16, kind="ExternalInput").ap()
    qks_ap = nc.dram_tensor("qks", [2, P, 1], F32, kind="ExternalInput").ap()
    qkt_ap = nc.dram_tensor("qkt", [2, P, 1], F32, kind="ExternalInput").ap()
    pegd_ap = nc.dram_tensor("pegd", [2, 9, P, P], BF16, kind="ExternalInput").ap()
    p1t_ap = nc.dram_tensor("p1t", [2, P, C], BF16, kind="ExternalInput").ap()
    pbs_ap = nc.dram_tensor("pbs", [2, P, 1], F32, kind="ExternalInput").ap()
    pbt_ap = nc.dram_tensor("pbt", [2, P, 1], F32, kind="ExternalInput").ap()
    p2t_ap = nc.dram_tensor("p2t", [2, P, HID], BF16, kind="ExternalInput").ap()
    p3t_ap = nc.dram_tensor("p3t", [8, P, C], BF16, kind="ExternalInput").ap()
    out_ap = nc.dram_tensor("out", [2, P, N], F32, kind="ExternalOutput").ap()
    dbg_aps = {}
    if dbg:
        for nm in ("dq", "dk", "dv0", "dv1", "dz0", "dz1"):
            dbg_aps[nm] = nc.dram_tensor(nm, [P, N], BF16, kind="ExternalOutput").ap()
        dbg_aps["dvT"] = nc.dram_tensor("dvT", [P, MT * 257], BF16, kind="ExternalOutput").ap()

    with tile.TileContext(nc) as tc, ExitStack() as ctx:
        pers = ctx.enter_context(tc.tile_pool(name="pers", bufs=1))
        pmm = ctx.enter_context(tc.tile_pool(name="pmm", bufs=4, space="PSUM"))
        pzt = ctx.enter_context(tc.tile_pool(name="pzt", bufs=4, space="PSUM"))

        # ---- persistent SBUF tensors ----
        x_sb = [pers.tile([P, N], F32, name=f"x{ct}") for ct in range(2)]
        xpad_bf = [pers.tile([P, PADN], BF16, name=f"xpadbf{ct}") for ct in range(2)]
        qk_sb = [pers.tile([P, N], BF16, name=f"qk{ct}") for ct in range(2)]  # q, k
        v_sb = [pers.tile([P, N], BF16, name=f"v{ct}") for ct in range(2)]
        vT = pers.tile([P, MT * 257], FP8, name="vT")
        z_sb = [pers.tile([P, N], BF16, name=f"z{ct}") for ct in range(2)]

        qkd_sb = pers.tile([P, 18 * P], BF16, name="qkd")
        qks_sb = [pers.tile([P, 1], F32, name=f"qks{ct}") for ct in range(2)]
        qkt_sb = [pers.tile([P, 1], F32, name=f"qkt{ct}") for ct in range(2)]
        pegd_sb = pers.tile([P, 18 * P], BF16, name="pegd")
        p1t_sb = pers.tile([P, 2 * C], BF16, name="p1t")
        pbs_sb = [pers.tile([P, 1], F32, name=f"pbs{ct}") for ct in range(2)]
        pbt_sb = [pers.tile([P, 1], F32, name=f"pbt{ct}") for ct in range(2)]
        p2t_sb = pers.tile([P, 2 * HID], BF16, name="p2t")
        p3t_sb = pers.tile([P, 8 * C], BF16, name="p3t")

        xp3b = [t.rearrange("p (h w) -> p h w", h=HP, w=HP) for t in xpad_bf]
        vT3 = vT.rearrange("p (m c) -> p m c", m=MT, c=257)

        # ---- load inputs / weights ----
        for ct in range(2):
            xb3 = xp3b[ct]
            nc.gpsimd.memset(xb3[:, 0, :], 0.0)
            nc.gpsimd.memset(xb3[:, 65, :], 0.0)
            nc.gpsimd.memset(xb3[:, 1:65, 0], 0.0)
            nc.gpsimd.memset(xb3[:, 1:65, 65], 0.0)
            for half in range(2):
                nc.sync.dma_start(
                    x_sb[ct][:, half * (N // 2):(half + 1) * (N // 2)],
                    x_ap[ct].rearrange("p h w -> p (h w)")[:, half * (N // 2):
                                                          (half + 1) * (N // 2)])
            nc.sync.dma_start(qks_sb[ct][:], qks_ap[ct])
            nc.sync.dma_start(qkt_sb[ct][:], qkt_ap[ct])
            for t in range(9):
                nc.sync.dma_start(
                    pegd_sb[:, (ct * 9 + t) * P:(ct * 9 + t + 1) * P],
                    pegd_ap[ct, t])
                nc.sync.dma_start(
                    qkd_sb[:, (ct * 9 + t) * P:(ct * 9 + t + 1) * P],
                    qkd_ap[ct, t])
        def load_proj_weights():
            for ct in range(2):
                nc.sync.dma_start(pbs_sb[ct][:], pbs_ap[ct])
                nc.sync.dma_start(pbt_sb[ct][:], pbt_ap[ct])
                nc.sync.dma_start(p1t_sb[:, ct * C:(ct + 1) * C], p1t_ap[ct])
                nc.sync.dma_start(p2t_sb[:, ct * HID:(ct + 1) * HID], p2t_ap[ct])
            for kt in range(8):
                nc.sync.dma_start(p3t_sb[:, kt * C:(kt + 1) * C], p3t_ap[kt])

        # constants / pools used by the compute body
        att_pool = ctx.enter_context(tc.tile_pool(name="att", bufs=3))
        epi_pool = ctx.enter_context(tc.tile_pool(name="epi", bufs=4))
        proj_pool = ctx.enter_context(tc.tile_pool(name="proj", bufs=4))
        out_pool = ctx.enter_context(tc.tile_pool(name="outp", bufs=4))
        ident = pers.tile([P, P], BF16, name="ident")
        make_identity(nc, ident)
        nc.gpsimd.memset(vT3[:, :, 256:257], 1.0)

        def emit_body():
            # bf16 padded input (conv rhs); borders stay zero from the memset
            for ct in range(2):
                for half in range(2):
                    nc.vector.tensor_copy(
                        xp3b[ct][:, 1 + half * 32:1 + (half + 1) * 32, 1:65],
                        x_sb[ct].rearrange("p (h w) -> p h w", h=H, w=W)
                        [:, half * 32:(half + 1) * 32, :])

            # ---- PEG conv (V branch) on PE ----
            for ct in range(2):
                def evict_v(nb, vp, ct=ct):
                    nc.scalar.copy(v_sb[ct][:, nb * NT:(nb + 1) * NT], vp[:])
                _conv_pe(nc, pmm, pegd_sb, ct * 9, xp3b[ct], evict_v)

            # ---- QK conv interleaved with vT transposes (PE works while DVE
            # evicts the transposed tiles) ----
            def emit_vt_group(g):
                # 8 transposes per group g in 0..7
                for i in range(8):
                    mi, ct = divmod(g * 8 + i, 2)
                    vtp = pzt.tile([P, P], BF16, name="vtp",
                                   tag=f"zt{(g * 8 + i) % 4}", bufs=1)
                    nc.tensor.transpose(
                        vtp[:], v_sb[ct][:, mi * P:(mi + 1) * P], ident[:])
                    nc.vector.tensor_copy(
                        vT3[:, mi, ct * P:(ct + 1) * P], vtp[:])

            g = 0
            for ct in range(2):
                def evict_qk(nb, vp, ct=ct):
                    nc.scalar.activation(
                        qk_sb[ct][:, nb * NT:(nb + 1) * NT], vp[:], AF.Silu,
                        bias=qkt_sb[ct][:], scale=qks_sb[ct][:])
                for nb in range(NB):
                    if nb % 2 == 0:
                        emit_vt_group(g)
                        g += 1
                    vp = pmm.tile([P, NT], F32, name="convp", tag="mm")
                    ti = 0
                    for dy in range(3):
                        for dx in range(3):
                            win = xp3b[ct][:, dy + nb * 8:dy + nb * 8 + 8,
                                           dx:dx + W]
                            nc.tensor.matmul(
                                vp[:], qkd_sb[:, (ct * 9 + ti) * P:
                                              (ct * 9 + ti + 1) * P],
                                win, start=(ti == 0), stop=(ti == 8))
                            ti += 1
                    evict_qk(nb, vp)

            # ---- attention (software-pipelined: lg of pair i+1 before zt of i) ----
            load_proj_weights()
            q, k = qk_sb[0], qk_sb[1]
            NPAIR = MT // 2

            def emit_lg_pair(nt, mp):
                tiles = []
                for h in range(2):
                    mi = 2 * mp + h
                    lg = pmm.tile([P, NT], F32, name="lg", tag="mm")
                    nc.tensor.matmul(
                        lg[:], k[:, mi * P:(mi + 1) * P],
                        q[:, nt * NT:(nt + 1) * NT], start=True, stop=True)
                    tiles.append(lg)
                return tiles

            def pair_seq():
                for nt in range(NB):
                    for mp in range(NPAIR):
                        yield nt, mp

            seq = list(pair_seq())
            pend = {}
            pend[seq[0]] = emit_lg_pair(*seq[0])
            pend[seq[1]] = emit_lg_pair(*seq[1])
            for idx, (nt, mp) in enumerate(seq):
                if mp == 0:
                    zt = [pzt.tile([P, 257], F32, name=f"ztp{j}", tag=f"zt{j}",
                                   bufs=1) for j in range(4)]
                lg2 = pend.pop((nt, mp))
                et = att_pool.tile([P, 2 * NT], FP8, name="et", tag="et", bufs=3)
                # two halves of the pair exp'd concurrently on ACT and DVE
                for h in range(2):
                    dve = mp >= 2 and h == mp % 2
                    if dve:
                        nc.vector._custom_dve(
                            _EXP_OP, out=et[:, h * NT:(h + 1) * NT],
                            in0=lg2[h][:],
                            s0=EXP_C[0], s1=EXP_C[1], imm2=EXP_C[2])
                    else:
                        nc.scalar.activation(et[:, h * NT:(h + 1) * NT],
                                             lg2[h][:], AF.Exp, scale=1.0 / 16.0)
                if idx + 2 < len(seq):
                    pend[seq[idx + 2]] = emit_lg_pair(*seq[idx + 2])
                et3 = et.rearrange("p (h n) -> p h n", h=2, n=NT)
                for j in range(4):
                    nc.tensor.matmul(
                        zt[j][:],
                        et3[:, :, j * P:(j + 1) * P],
                        vT3[:, 2 * mp:2 * mp + 2, :],
                        start=(mp == 0), stop=(mp == NPAIR - 1),
                        perf_mode=mybir.MatmulPerfMode.DoubleRow)
                if mp == NPAIR - 1:
                    for j in range(4):
                        recip = epi_pool.tile([P, 1], F32, name="recip",
                                              tag="recip")
                        nc.vector.reciprocal(recip[:], zt[j][:, 256:257])
                        zn = epi_pool.tile([P, C], BF16, name="zn", tag="zn")
                        nc.vector.tensor_scalar(zn[:], zt[j][:, :C], recip[:],
                                                None, ALU.mult)
                        for ct in range(2):
                            tp2 = pmm.tile([P, P], BF16, name="tp2", tag="mm")
                            nc.tensor.transpose(tp2[:], zn[:, ct * P:(ct + 1) * P],
                                                ident[:])
                            nc.vector.tensor_copy(
                                z_sb[ct][:, nt * NT + j * P:
                                          nt * NT + (j + 1) * P], tp2[:])

            if dbg:
                nc.sync.dma_start(dbg_aps["dq"][:], qk_sb[0][:])
                nc.sync.dma_start(dbg_aps["dk"][:], qk_sb[1][:])
                nc.sync.dma_start(dbg_aps["dz0"][:], z_sb[0][:])
                nc.sync.dma_start(dbg_aps["dz1"][:], z_sb[1][:])
                nc.sync.dma_start(dbg_aps["dvT"][:], vT[:])

            # ---- projection MLP: p1 -> BN -> p2 -> SiLU -> p3(*gamma) + x ----
            for nt in range(NB):
                ns = slice(nt * NT, (nt + 1) * NT)
                h1 = []
                for ot in range(2):
                    h1p = pzt.tile([P, NT], F32, name="h1p", tag=f"zt{ot}", bufs=1)
                    for kt in range(2):
                        nc.tensor.matmul(
                            h1p[:],
                            p1t_sb[:, kt * C + ot * P:kt * C + (ot + 1) * P],
                            z_sb[kt][:, ns], start=(kt == 0), stop=(kt == 1))
                    h1t = proj_pool.tile([P, NT], BF16, name="h1t", tag="h1")
                    nc.vector.tensor_scalar(h1t[:], h1p[:], pbs_sb[ot][:],
                                            pbt_sb[ot][:], ALU.mult, ALU.add)
                    h1.append(h1t)
                h2 = []
                for ht in range(8):
                    h2p = pmm.tile([P, NT], F32, name="h2p", tag="mm")
                    for kt in range(2):
                        nc.tensor.matmul(
                            h2p[:],
                            p2t_sb[:, kt * HID + ht * P:kt * HID + (ht + 1) * P],
                            h1[kt][:], start=(kt == 0), stop=(kt == 1))
                    h2t = proj_pool.tile([P, NT], BF16, name="h2t", tag="h2",
                                         bufs=10)
                    nc.scalar.activation(h2t[:], h2p[:], AF.Silu)
                    h2.append(h2t)
                for ot in range(2):
                    zfp = pzt.tile([P, NT], F32, name="zfp", tag=f"zt{ot + 2}",
                                   bufs=1)
                    for kt in range(8):
                        nc.tensor.matmul(
                            zfp[:],
                            p3t_sb[:, kt * C + ot * P:kt * C + (ot + 1) * P],
                            h2[kt][:], start=(kt == 0), stop=(kt == 7))
                    ob = out_pool.tile([P, NT], F32, name="ob", tag="ob")
                    nc.vector.tensor_tensor(ob[:], zfp[:], x_sb[ot][:, ns], ALU.add)
                    nc.sync.dma_start(out_ap[ot][:, ns], ob[:])

        for _rep in range(reps):
            emit_body()

    nc.finalize()
    return nc


def _prep_inputs(x, qk_w, qk_g, qk_b, qk_m, qk_v, peg_w,
                 p1_w, pb_g, pb_b, pb_m, pb_v, p2_w, p3_w, gamma):
    f32 = np.float32
    bf16 = ml_dtypes.bfloat16

    def diag_taps(w9):
        d = np.zeros((2, 9, P, P), f32)
        idx = np.arange(P)
        for ct in range(2):
            for t in range(9):
                d[ct, t, idx, idx] = w9[ct * P:(ct + 1) * P, t]
        return d

    qks = (qk_g / np.sqrt(qk_v + EPS)).astype(f32)
    qkt = (qk_b - qk_m * qks).astype(f32)
    qkd = diag_taps(np.asarray(qk_w, f32).reshape(C, 9))

    pegw = np.asarray(peg_w, f32).reshape(C, 9).copy()
    pegw[:, 4] += 1.0  # fold +x residual into center tap
    pegd = diag_taps(pegw)

    pbs = (pb_g / np.sqrt(pb_v + EPS)).astype(f32)
    pbt = (pb_b - pb_m * pbs).astype(f32)

    p1t = np.ascontiguousarray(np.asarray(p1_w, f32).T).reshape(2, P, C)
    p2t = np.ascontiguousarray(np.asarray(p2_w, f32).T).reshape(2, P, HID)
    p3g = np.asarray(p3_w, f32) * np.asarray(gamma, f32)[:, None]
    p3t = np.ascontiguousarray(p3g.T).reshape(8, P, C)

    shared = {
        "qkd": qkd.astype(bf16),
        "qks": qks.reshape(2, P, 1).astype(f32),
        "qkt": qkt.reshape(2, P, 1).astype(f32),
        "pegd": pegd.astype(bf16),
        "p1t": p1t.astype(bf16),
        "pbs": pbs.reshape(2, P, 1).astype(f32),
        "pbt": pbt.reshape(2, P, 1).astype(f32),
        "p2t": p2t.astype(bf16),
        "p3t": p3t.astype(bf16),
    }
    xs = np.asarray(x, f32).reshape(8, 2, P, H, W)
    return [dict(shared, x=np.ascontiguousarray(xs[i])) for i in range(N_CORES)]


def kernel(**inputs):
    if "nc" not in _cache:
        _cache["nc"] = _build_program()
    nc = _cache["nc"]
    in_maps = _prep_inputs(**inputs)
    res = run_bass_kernel_spmd(nc, in_maps, list(range(N_CORES)))
    _cache["last_result"] = res
    out = np.stack([res.results[i]["out"].reshape(C, H, W)
                    for i in range(N_CORES)])
    return out.astype(np.float32)


# revision 25
# speedup vs baseline: 1.1743x; 1.1042x over previous
"""DSA single-head attention block (dwconv QK + PEG V + attention + MLP) on 8 trn2 cores.

Sharding: data-parallel over batch (8 images -> 8 cores), weights replicated.
Self-contained: hardcodes shapes B=8, C=256, H=W=64, hidden=1024.

Per-core pipeline:
  - depthwise 3x3 convs (QK and PEG+residual) on the tensor engine via
    per-tap diagonal matrices accumulated in PSUM (bf16)
  - attention computed as logitsT[m,n] = k^T q tiles; exp on ACT straight out
    of PSUM (1/sqrt(C) folded into the activation scale); zT[n,c] accumulated
    with an extra ones column in vT producing the softmax denominators free
  - per-partition normalize, DMA-transpose back to [c,n]
  - MLP p1 -> BN -> p2 -> SiLU -> p3(*gamma) in bf16, f32 residual add
"""

import os
import sys

for _p in ("/opt/trn_rl_repo", os.path.expanduser("~/.axon_site/_ro/trn_rl_repo")):
    if os.path.isdir(_p) and _p not in sys.path:
        sys.path.insert(0, _p)

from contextlib import ExitStack

import ml_dtypes
import numpy as np

import concourse.bass as bass
import concourse.tile as tile
from concourse import bacc, mybir
from concourse.bass_utils import run_bass_kernel_spmd
from concourse.masks import make_identity
from concourse import dve_ops as _dvo
from concourse.dve_spec import Spec as _DveSpec, Src0, Src1, C0, C1, C2, One
from concourse.dve_spec import lower as _dve_lower
from concourse.dve_uop import DveOpSpec as _DveOpSpec

# exp(y/16) = (exp(y/32))^2 ~= p(y)^2 with p = degree-3 Taylor of exp(y/32);
# rel err < 2e-3 for |y| <= 11, far below the fp8e4 output rounding. The
# squared form needs only the 3 wired scalar slots (Src1 [P,1] broadcast
# crashes this silicon path).
EXP_C = (1.0 / 32, 1.0 / 2048, 1.0 / 196608)


def _register_exp_op():
    name = "EXP16SQ_ANT"
    for o in _dvo.OPS:
        if o.name == name:
            return o

    def _exp_ref(in0, in1, c0, c1, c2):
        x = in0.astype("float32")
        t = 1.0 + x * (c0 + x * (c1 + x * c2))
        return t * t

    _t = One + Src0 * (C0 + Src0 * (C1 + Src0 * C2))
    spec = _DveSpec(body=_t * _t, reference=_exp_ref)
    op = _dvo.DveOp(name, spec, subdim=False, uops_sha={})
    _dvo.OPS.append(op)
    row = _dvo._CUSTOM_DVE_ROW_BASE + len(_dvo.OPS) - 1
    assert row < 0x20
    _dvo._SUB_OPCODE_FOR_NAME[name] = row
    _dvo.CUSTOM_DVE_SPECS[name] = spec
    for ver in ("v3", "v4"):
        r = _DveOpSpec(name=name, opcode=row, uops=_dve_lower(spec, ver=ver),
                       rd1_en=False)
        op.uops_sha[ver] = r.sha(ver)
    return op


_EXP_OP = _register_exp_op()

F32 = mybir.dt.float32
BF16 = mybir.dt.bfloat16
FP8 = mybir.dt.float8e4
AF = mybir.ActivationFunctionType
ALU = mybir.AluOpType

P = 128          # partitions
C = 256          # channels
H = W = 64
N = H * W        # 4096 tokens
HP = H + 2       # padded spatial
PADN = HP * HP   # 4356
NT = 512         # token block (q columns per attention outer step)
NB = N // NT     # 8
MT = 32          # number of 128-wide m tiles
HID = 1024
EPS = 1e-5
N_CORES = 8

_cache = {}


def _conv_pe(nc, pmm, diag_sb, base, xp3b, dst_evict):
    """Depthwise 3x3 conv on PE: 9 diag-matmuls accumulated in PSUM per
    512-token block; dst_evict(nb, psum_ap) consumes each block."""
    for nb in range(NB):
        vp = pmm.tile([P, NT], F32, name="convp", tag="mm")
        ti = 0
        for dy in range(3):
            for dx in range(3):
                win = xp3b[:, dy + nb * 8:dy + nb * 8 + 8, dx:dx + W]
                nc.tensor.matmul(
                    vp[:], diag_sb[:, (base + ti) * P:(base + ti + 1) * P],
                    win, start=(ti == 0), stop=(ti == 8))
                ti += 1
        dst_evict(nb, vp)


def _build_program(dbg=False, reps=1):
    nc = bacc.Bacc("TRN2", target_bir_lowering=False, debug=False,
                   num_devices=N_CORES)

    x_ap = nc.dram_tensor("x", [2, P, H, W], F32, kind="ExternalInput").ap()
    qkd_ap = nc.dram_tensor("qkd", [2, 9, P, P], BF16, kind="ExternalInput").ap()
    qks_ap = nc.dram_tensor("qks", [2, P, 1], F32, kind="ExternalInput").ap()
    qkt_ap = nc.dram_tensor("qkt", [2, P, 1], F32, kind="ExternalInput").ap()
    pegd_ap = nc.dram_tensor("pegd", [2, 9, P, P], BF16, kind="ExternalInput").ap()
    p1t_ap = nc.dram_tensor("p1t", [2, P, C], BF16, kind="ExternalInput").ap()
    pbs_ap = nc.dram_tensor("pbs", [2, P, 1], F32, kind="ExternalInput").ap()
    pbt_ap = nc.dram_tensor("pbt", [2, P, 1], F32, kind="ExternalInput").ap()
    p2t_ap = nc.dram_tensor("p2t", [2, P, HID], FP8, kind="ExternalInput").ap()
    p3t_ap = nc.dram_tensor("p3t", [8, P, C], FP8, kind="ExternalInput").ap()
    out_ap = nc.dram_tensor("out", [2, P, N], F32, kind="ExternalOutput").ap()
    dbg_aps = {}
    if dbg:
        for nm in ("dq", "dk", "dv0", "dv1", "dz0", "dz1"):
            dbg_aps[nm] = nc.dram_tensor(nm, [P, N], BF16, kind="ExternalOutput").ap()
        dbg_aps["dvT"] = nc.dram_tensor("dvT", [P, MT * 257], BF16, kind="ExternalOutput").ap()

    with tile.TileContext(nc) as tc, ExitStack() as ctx:
        pers = ctx.enter_context(tc.tile_pool(name="pers", bufs=1))
        pmm = ctx.enter_context(tc.tile_pool(name="pmm", bufs=4, space="PSUM"))
        pzt = ctx.enter_context(tc.tile_pool(name="pzt", bufs=4, space="PSUM"))

        # ---- persistent SBUF tensors ----
        x_sb = [pers.tile([P, N], F32, name=f"x{ct}") for ct in range(2)]
        xpad_bf = [pers.tile([P, PADN], BF16, name=f"xpadbf{ct}") for ct in range(2)]
        qk_sb = [pers.tile([P, N], BF16, name=f"qk{ct}") for ct in range(2)]  # q, k
        v_sb = [pers.tile([P, N], BF16, name=f"v{ct}") for ct in range(2)]
        vT = pers.tile([P, MT * 257], FP8, name="vT")
        z_sb = [pers.tile([P, N], BF16, name=f"z{ct}") for ct in range(2)]

        qkd_sb = pers.tile([P, 18 * P], BF16, name="qkd")
        qks_sb = [pers.tile([P, 1], F32, name=f"qks{ct}") for ct in range(2)]
        qkt_sb = [pers.tile([P, 1], F32, name=f"qkt{ct}") for ct in range(2)]
        pegd_sb = pers.tile([P, 18 * P], BF16, name="pegd")
        p1t_sb = pers.tile([P, 2 * C], BF16, name="p1t")
        pbs_sb = [pers.tile([P, 1], F32, name=f"pbs{ct}") for ct in range(2)]
        pbt_sb = [pers.tile([P, 1], F32, name=f"pbt{ct}") for ct in range(2)]
        p2t_sb = pers.tile([P, 2 * HID], FP8, name="p2t")
        p3t_sb = pers.tile([P, 8 * C], FP8, name="p3t")

        xp3b = [t.rearrange("p (h w) -> p h w", h=HP, w=HP) for t in xpad_bf]
        vT3 = vT.rearrange("p (m c) -> p m c", m=MT, c=257)

        # ---- load inputs / weights ----
        for ct in range(2):
            xb3 = xp3b[ct]
            nc.gpsimd.memset(xb3[:, 0, :], 0.0)
            nc.gpsimd.memset(xb3[:, 65, :], 0.0)
            nc.gpsimd.memset(xb3[:, 1:65, 0], 0.0)
            nc.gpsimd.memset(xb3[:, 1:65, 65], 0.0)
            for half in range(2):
                nc.sync.dma_start(
                    x_sb[ct][:, half * (N // 2):(half + 1) * (N // 2)],
                    x_ap[ct].rearrange("p h w -> p (h w)")[:, half * (N // 2):
                                                          (half + 1) * (N // 2)])
            nc.sync.dma_start(qks_sb[ct][:], qks_ap[ct])
            nc.sync.dma_start(qkt_sb[ct][:], qkt_ap[ct])
            for t in range(9):
                nc.sync.dma_start(
                    pegd_sb[:, (ct * 9 + t) * P:(ct * 9 + t + 1) * P],
                    pegd_ap[ct, t])
                nc.sync.dma_start(
                    qkd_sb[:, (ct * 9 + t) * P:(ct * 9 + t + 1) * P],
                    qkd_ap[ct, t])
        def load_proj_weights():
            for ct in range(2):
                nc.sync.dma_start(pbs_sb[ct][:], pbs_ap[ct])
                nc.sync.dma_start(pbt_sb[ct][:], pbt_ap[ct])
                nc.sync.dma_start(p1t_sb[:, ct * C:(ct + 1) * C], p1t_ap[ct])
                nc.sync.dma_start(p2t_sb[:, ct * HID:(ct + 1) * HID], p2t_ap[ct])
            for kt in range(8):
                nc.sync.dma_start(p3t_sb[:, kt * C:(kt + 1) * C], p3t_ap[kt])

        # constants / pools used by the compute body
        att_pool = ctx.enter_context(tc.tile_pool(name="att", bufs=3))
        epi_pool = ctx.enter_context(tc.tile_pool(name="epi", bufs=4))
        proj_pool = ctx.enter_context(tc.tile_pool(name="proj", bufs=4))
        out_pool = ctx.enter_context(tc.tile_pool(name="outp", bufs=4))
        ident = pers.tile([P, P], BF16, name="ident")
        make_identity(nc, ident)
        nc.gpsimd.memset(vT3[:, :, 256:257], 1.0)

        def emit_body():
            # bf16 padded input (conv rhs); borders stay zero from the memset
            for ct in range(2):
                for half in range(2):
                    nc.vector.tensor_copy(
                        xp3b[ct][:, 1 + half * 32:1 + (half + 1) * 32, 1:65],
                        x_sb[ct].rearrange("p (h w) -> p h w", h=H, w=W)
                        [:, half * 32:(half + 1) * 32, :])

            # ---- PEG conv (V branch) on PE ----
            for ct in range(2):
                def evict_v(nb, vp, ct=ct):
                    nc.scalar.copy(v_sb[ct][:, nb * NT:(nb + 1) * NT], vp[:])
                _conv_pe(nc, pmm, pegd_sb, ct * 9, xp3b[ct], evict_v)

            # ---- QK conv interleaved with vT transposes (PE works while DVE
            # evicts the transposed tiles) ----
            def emit_vt_group(g):
                # 8 transposes per group g in 0..7
                for i in range(8):
                    mi, ct = divmod(g * 8 + i, 2)
                    vtp = pzt.tile([P, P], BF16, name="vtp",
                                   tag=f"zt{(g * 8 + i) % 4}", bufs=1)
                    nc.tensor.transpose(
                        vtp[:], v_sb[ct][:, mi * P:(mi + 1) * P], ident[:])
                    nc.vector.tensor_copy(
                        vT3[:, mi, ct * P:(ct + 1) * P], vtp[:])

            g = 0
            for ct in range(2):
                def evict_qk(nb, vp, ct=ct):
                    nc.scalar.activation(
                        qk_sb[ct][:, nb * NT:(nb + 1) * NT], vp[:], AF.Silu,
                        bias=qkt_sb[ct][:], scale=qks_sb[ct][:])
                for nb in range(NB):
                    if nb % 2 == 0:
                        emit_vt_group(g)
                        g += 1
                    vp = pmm.tile([P, NT], F32, name="convp", tag="mm")
                    ti = 0
                    for dy in range(3):
                        for dx in range(3):
                            win = xp3b[ct][:, dy + nb * 8:dy + nb * 8 + 8,
                                           dx:dx + W]
                            nc.tensor.matmul(
                                vp[:], qkd_sb[:, (ct * 9 + ti) * P:
                                              (ct * 9 + ti + 1) * P],
                                win, start=(ti == 0), stop=(ti == 8))
                            ti += 1
                    evict_qk(nb, vp)

            # ---- attention (software-pipelined: lg of pair i+1 before zt of i) ----
            load_proj_weights()
            q, k = qk_sb[0], qk_sb[1]
            NPAIR = MT // 2

            def emit_lg_pair(nt, mp):
                tiles = []
                for h in range(2):
                    mi = 2 * mp + h
                    lg = pmm.tile([P, NT], F32, name="lg", tag="mm")
                    nc.tensor.matmul(
                        lg[:], k[:, mi * P:(mi + 1) * P],
                        q[:, nt * NT:(nt + 1) * NT], start=True, stop=True)
                    tiles.append(lg)
                return tiles

            def pair_seq():
                for nt in range(NB):
                    for mp in range(NPAIR):
                        yield nt, mp

            seq = list(pair_seq())
            pend = {}
            pend[seq[0]] = emit_lg_pair(*seq[0])
            pend[seq[1]] = emit_lg_pair(*seq[1])
            for idx, (nt, mp) in enumerate(seq):
                if mp == 0:
                    zt = [pzt.tile([P, 257], F32, name=f"ztp{j}", tag=f"zt{j}",
                                   bufs=1) for j in range(4)]
                lg2 = pend.pop((nt, mp))
                et = att_pool.tile([P, 2 * NT], FP8, name="et", tag="et", bufs=3)
                # two halves of the pair exp'd concurrently on ACT and DVE
                for h in range(2):
                    dve = mp >= 2 and h == mp % 2
                    if dve:
                        nc.vector._custom_dve(
                            _EXP_OP, out=et[:, h * NT:(h + 1) * NT],
                            in0=lg2[h][:],
                            s0=EXP_C[0], s1=EXP_C[1], imm2=EXP_C[2])
                    else:
                        nc.scalar.activation(et[:, h * NT:(h + 1) * NT],
                                             lg2[h][:], AF.Exp, scale=1.0 / 16.0)
                if idx + 2 < len(seq):
                    pend[seq[idx + 2]] = emit_lg_pair(*seq[idx + 2])
                et3 = et.rearrange("p (h n) -> p h n", h=2, n=NT)
                for j in range(4):
                    nc.tensor.matmul(
                        zt[j][:],
                        et3[:, :, j * P:(j + 1) * P],
                        vT3[:, 2 * mp:2 * mp + 2, :],
                        start=(mp == 0), stop=(mp == NPAIR - 1),
                        perf_mode=mybir.MatmulPerfMode.DoubleRow)
                if mp == NPAIR - 1:
                    for j in range(4):
                        recip = epi_pool.tile([P, 1], F32, name="recip",
                                              tag="recip")
                        nc.vector.reciprocal(recip[:], zt[j][:, 256:257])
                        zn = epi_pool.tile([P, C], BF16, name="zn", tag="zn")
                        nc.vector.tensor_scalar(zn[:], zt[j][:, :C], recip[:],
                                                None, ALU.mult)
                        for ct in range(2):
                            tp2 = pmm.tile([P, P], BF16, name="tp2", tag="mm")
                            nc.tensor.transpose(tp2[:], zn[:, ct * P:(ct + 1) * P],
                                                ident[:])
                            nc.vector.tensor_copy(
                                z_sb[ct][:, nt * NT + j * P:
                                          nt * NT + (j + 1) * P], tp2[:])

            if dbg:
                nc.sync.dma_start(dbg_aps["dq"][:], qk_sb[0][:])
                nc.sync.dma_start(dbg_aps["dk"][:], qk_sb[1][:])
                nc.sync.dma_start(dbg_aps["dz0"][:], z_sb[0][:])
                nc.sync.dma_start(dbg_aps["dz1"][:], z_sb[1][:])
                nc.sync.dma_start(dbg_aps["dvT"][:], vT[:])

            # ---- projection MLP: p1 -> BN -> p2 -> SiLU -> p3(*gamma) + x ----
            for nt in range(NB):
                ns = slice(nt * NT, (nt + 1) * NT)
                h1pair = proj_pool.tile([P, 2 * NT], FP8, name="h1pair",
                                        tag="h1", bufs=3)
                for ot in range(2):
                    h1p = pzt.tile([P, NT], F32, name="h1p", tag=f"zt{ot}", bufs=1)
                    for kt in range(2):
                        nc.tensor.matmul(
                            h1p[:],
                            p1t_sb[:, kt * C + ot * P:kt * C + (ot + 1) * P],
                            z_sb[kt][:, ns], start=(kt == 0), stop=(kt == 1))
                    nc.vector.tensor_scalar(h1pair[:, ot * NT:(ot + 1) * NT],
                                            h1p[:], pbs_sb[ot][:],
                                            pbt_sb[ot][:], ALU.mult, ALU.add)
                h1pr = h1pair.rearrange("p (a b) -> p a b", a=2, b=NT)
                p2pr = p2t_sb.rearrange("p (a b) -> p a b", a=2, b=HID)
                h2 = []
                for g in range(4):
                    h2t = proj_pool.tile([P, 2 * NT], FP8, name="h2t", tag="h2",
                                         bufs=6)
                    h2.append(h2t)
                for ht in range(8):
                    h2p = pmm.tile([P, NT], F32, name="h2p", tag="mm")
                    nc.tensor.matmul(
                        h2p[:], p2pr[:, :, ht * P:(ht + 1) * P], h1pr[:],
                        start=True, stop=True,
                        perf_mode=mybir.MatmulPerfMode.DoubleRow)
                    nc.scalar.activation(
                        h2[ht // 2][:, (ht % 2) * NT:(ht % 2 + 1) * NT],
                        h2p[:], AF.Silu)
                for ot in range(2):
                    zfp = pzt.tile([P, NT], F32, name="zfp", tag=f"zt{ot + 2}",
                                   bufs=1)
                    for g in range(4):
                        p3pr = p3t_sb[:, 2 * g * C:(2 * g + 2) * C].rearrange(
                            "p (a b) -> p a b", a=2, b=C)
                        h2pr = h2[g].rearrange("p (a b) -> p a b", a=2, b=NT)
                        nc.tensor.matmul(
                            zfp[:], p3pr[:, :, ot * P:(ot + 1) * P], h2pr[:],
                            start=(g == 0), stop=(g == 3),
                            perf_mode=mybir.MatmulPerfMode.DoubleRow)
                    ob = out_pool.tile([P, NT], F32, name="ob", tag="ob")
                    nc.vector.tensor_tensor(ob[:], zfp[:], x_sb[ot][:, ns], ALU.add)
                    nc.sync.dma_start(out_ap[ot][:, ns], ob[:])

        for _rep in range(reps):
            emit_body()

    nc.finalize()
    return nc


def _prep_inputs(x, qk_w, qk_g, qk_b, qk_m, qk_v, peg_w,
                 p1_w, pb_g, pb_b, pb_m, pb_v, p2_w, p3_w, gamma):
    f32 = np.float32
    bf16 = ml_dtypes.bfloat16
    fp8 = ml_dtypes.float8_e4m3

    def diag_taps(w9):
        d = np.zeros((2, 9, P, P), f32)
        idx = np.arange(P)
        for ct in range(2):
            for t in range(9):
                d[ct, t, idx, idx] = w9[ct * P:(ct + 1) * P, t]
        return d

    qks = (qk_g / np.sqrt(qk_v + EPS)).astype(f32)
    qkt = (qk_b - qk_m * qks).astype(f32)
    qkd = diag_taps(np.asarray(qk_w, f32).reshape(C, 9))

    pegw = np.asarray(peg_w, f32).reshape(C, 9).copy()
    pegw[:, 4] += 1.0  # fold +x residual into center tap
    pegd = diag_taps(pegw)

    pbs = (pb_g / np.sqrt(pb_v + EPS)).astype(f32)
    pbt = (pb_b - pb_m * pbs).astype(f32)

    p1t = np.ascontiguousarray(np.asarray(p1_w, f32).T).reshape(2, P, C)
    p2t = np.ascontiguousarray(np.asarray(p2_w, f32).T).reshape(2, P, HID)
    p3g = np.asarray(p3_w, f32) * np.asarray(gamma, f32)[:, None]
    p3t = np.ascontiguousarray(p3g.T).reshape(8, P, C)

    shared = {
        "qkd": qkd.astype(bf16),
        "qks": qks.reshape(2, P, 1).astype(f32),
        "qkt": qkt.reshape(2, P, 1).astype(f32),
        "pegd": pegd.astype(bf16),
        "p1t": p1t.astype(bf16),
        "pbs": pbs.reshape(2, P, 1).astype(f32),
        "pbt": pbt.reshape(2, P, 1).astype(f32),
        "p2t": p2t.astype(fp8),
        "p3t": p3t.astype(fp8),
    }
    xs = np.asarray(x, f32).reshape(8, 2, P, H, W)
    return [dict(shared, x=np.ascontiguousarray(xs[i])) for i in range(N_CORES)]


def kernel(**inputs):
    if "nc" not in _cache:
        _cache["nc"] = _build_program()
    nc = _cache["nc"]
    in_maps = _prep_inputs(**inputs)
    res = run_bass_kernel_spmd(nc, in_maps, list(range(N_CORES)))
    _cache["last_result"] = res
    out = np.stack([res.results[i]["out"].reshape(C, H, W)
                    for i in range(N_CORES)])
    return out.astype(np.float32)


# revision 27
# speedup vs baseline: 1.2016x; 1.0232x over previous
"""DSA single-head attention block (dwconv QK + PEG V + attention + MLP) on 8 trn2 cores.

Sharding: data-parallel over batch (8 images -> 8 cores), weights replicated.
Self-contained: hardcodes shapes B=8, C=256, H=W=64, hidden=1024.

Per-core pipeline:
  - depthwise 3x3 convs (QK and PEG+residual) on the tensor engine via
    per-tap diagonal matrices accumulated in PSUM (bf16)
  - attention computed as logitsT[m,n] = k^T q tiles; exp on ACT straight out
    of PSUM (1/sqrt(C) folded into the activation scale); zT[n,c] accumulated
    with an extra ones column in vT producing the softmax denominators free
  - per-partition normalize, DMA-transpose back to [c,n]
  - MLP p1 -> BN -> p2 -> SiLU -> p3(*gamma) in bf16, f32 residual add
"""

import os
import sys

for _p in ("/opt/trn_rl_repo", os.path.expanduser("~/.axon_site/_ro/trn_rl_repo")):
    if os.path.isdir(_p) and _p not in sys.path:
        sys.path.insert(0, _p)

from contextlib import ExitStack

import ml_dtypes
import numpy as np

import concourse.bass as bass
import concourse.tile as tile
from concourse import bacc, mybir
from concourse.bass_utils import run_bass_kernel_spmd
from concourse.masks import make_identity
from concourse import dve_ops as _dvo
from concourse.dve_spec import Spec as _DveSpec, Src0, Src1, C0, C1, C2, One
from concourse.dve_spec import lower as _dve_lower
from concourse.dve_uop import DveOpSpec as _DveOpSpec

# exp(y/16) = (exp(y/32))^2 ~= p(y)^2 with p = degree-3 Taylor of exp(y/32);
# rel err < 2e-3 for |y| <= 11, far below the fp8e4 output rounding. The
# squared form needs only the 3 wired scalar slots (Src1 [P,1] broadcast
# crashes this silicon path).
EXP_C = (1.0 / 32, 1.0 / 2048, 1.0 / 196608)


def _register_exp_op():
    name = "EXP16SQ_ANT"
    for o in _dvo.OPS:
        if o.name == name:
            return o

    def _exp_ref(in0, in1, c0, c1, c2):
        x = in0.astype("float32")
        t = 1.0 + x * (c0 + x * (c1 + x * c2))
        return t * t

    _t = One + Src0 * (C0 + Src0 * (C1 + Src0 * C2))
    spec = _DveSpec(body=_t * _t, reference=_exp_ref)
    op = _dvo.DveOp(name, spec, subdim=False, uops_sha={})
    _dvo.OPS.append(op)
    row = _dvo._CUSTOM_DVE_ROW_BASE + len(_dvo.OPS) - 1
    assert row < 0x20
    _dvo._SUB_OPCODE_FOR_NAME[name] = row
    _dvo.CUSTOM_DVE_SPECS[name] = spec
    for ver in ("v3", "v4"):
        r = _DveOpSpec(name=name, opcode=row, uops=_dve_lower(spec, ver=ver),
                       rd1_en=False)
        op.uops_sha[ver] = r.sha(ver)
    return op


_EXP_OP = _register_exp_op()

F32 = mybir.dt.float32
BF16 = mybir.dt.bfloat16
FP8 = mybir.dt.float8e4
AF = mybir.ActivationFunctionType
ALU = mybir.AluOpType

P = 128          # partitions
C = 256          # channels
H = W = 64
N = H * W        # 4096 tokens
HP = H + 2       # padded spatial
PADN = HP * HP   # 4356
NT = 512         # token block (q columns per attention outer step)
NB = N // NT     # 8
MT = 32          # number of 128-wide m tiles
HID = 1024
EPS = 1e-5
N_CORES = 8

_cache = {}


def _conv_pe(nc, pmm, diag_sb, base, xp3b, dst_evict):
    """Depthwise 3x3 conv on PE: 9 diag-matmuls accumulated in PSUM per
    512-token block; dst_evict(nb, psum_ap) consumes each block."""
    for nb in range(NB):
        vp = pmm.tile([P, NT], F32, name="convp", tag="mm")
        ti = 0
        for dy in range(3):
            for dx in range(3):
                win = xp3b[:, dy + nb * 8:dy + nb * 8 + 8, dx:dx + W]
                nc.tensor.matmul(
                    vp[:], diag_sb[:, (base + ti) * P:(base + ti + 1) * P],
                    win, start=(ti == 0), stop=(ti == 8))
                ti += 1
        dst_evict(nb, vp)


def _build_program(dbg=False, reps=1):
    nc = bacc.Bacc("TRN2", target_bir_lowering=False, debug=False,
                   num_devices=N_CORES)

    x_ap = nc.dram_tensor("x", [2, P, H, W], F32, kind="ExternalInput").ap()
    qkd_ap = nc.dram_tensor("qkd", [2, 9, P, P], BF16, kind="ExternalInput").ap()
    qks_ap = nc.dram_tensor("qks", [2, P, 1], F32, kind="ExternalInput").ap()
    qkt_ap = nc.dram_tensor("qkt", [2, P, 1], F32, kind="ExternalInput").ap()
    pegd_ap = nc.dram_tensor("pegd", [2, 9, P, P], BF16, kind="ExternalInput").ap()
    p1t_ap = nc.dram_tensor("p1t", [2, P, C], BF16, kind="ExternalInput").ap()
    pbs_ap = nc.dram_tensor("pbs", [2, P, 1], F32, kind="ExternalInput").ap()
    pbt_ap = nc.dram_tensor("pbt", [2, P, 1], F32, kind="ExternalInput").ap()
    p2t_ap = nc.dram_tensor("p2t", [2, P, HID], FP8, kind="ExternalInput").ap()
    p3t_ap = nc.dram_tensor("p3t", [8, P, C], FP8, kind="ExternalInput").ap()
    out_ap = nc.dram_tensor("out", [2, P, N], F32, kind="ExternalOutput").ap()
    dbg_aps = {}
    if dbg:
        for nm in ("dq", "dk", "dv0", "dv1", "dz0", "dz1"):
            dbg_aps[nm] = nc.dram_tensor(nm, [P, N], BF16, kind="ExternalOutput").ap()
        dbg_aps["dvT"] = nc.dram_tensor("dvT", [P, MT * 257], BF16, kind="ExternalOutput").ap()

    with tile.TileContext(nc) as tc, ExitStack() as ctx:
        pers = ctx.enter_context(tc.tile_pool(name="pers", bufs=1))
        pmm = ctx.enter_context(tc.tile_pool(name="pmm", bufs=4, space="PSUM"))
        pzt = ctx.enter_context(tc.tile_pool(name="pzt", bufs=4, space="PSUM"))

        # ---- persistent SBUF tensors ----
        x_sb = [pers.tile([P, N], F32, name=f"x{ct}") for ct in range(2)]
        xpad_bf = [pers.tile([P, PADN], BF16, name=f"xpadbf{ct}") for ct in range(2)]
        qk_sb = [pers.tile([P, N], BF16, name=f"qk{ct}") for ct in range(2)]  # q, k
        v_sb = [pers.tile([P, N], BF16, name=f"v{ct}") for ct in range(2)]
        vT = pers.tile([P, MT * 257], FP8, name="vT")
        z_sb = [pers.tile([P, N], BF16, name=f"z{ct}") for ct in range(2)]

        qkd_sb = pers.tile([P, 18 * P], BF16, name="qkd")
        qks_sb = [pers.tile([P, 1], F32, name=f"qks{ct}") for ct in range(2)]
        qkt_sb = [pers.tile([P, 1], F32, name=f"qkt{ct}") for ct in range(2)]
        pegd_sb = pers.tile([P, 18 * P], BF16, name="pegd")
        p1t_sb = pers.tile([P, 2 * C], BF16, name="p1t")
        pbs_sb = [pers.tile([P, 1], F32, name=f"pbs{ct}") for ct in range(2)]
        pbt_sb = [pers.tile([P, 1], F32, name=f"pbt{ct}") for ct in range(2)]
        p2t_sb = pers.tile([P, 2 * HID], FP8, name="p2t")
        p3t_sb = pers.tile([P, 8 * C], FP8, name="p3t")

        xp3b = [t.rearrange("p (h w) -> p h w", h=HP, w=HP) for t in xpad_bf]
        vT3 = vT.rearrange("p (m c) -> p m c", m=MT, c=257)

        # ---- load inputs / weights ----
        for ct in range(2):
            xb3 = xp3b[ct]
            nc.gpsimd.memset(xb3[:, 0, :], 0.0)
            nc.gpsimd.memset(xb3[:, 65, :], 0.0)
            nc.gpsimd.memset(xb3[:, 1:65, 0], 0.0)
            nc.gpsimd.memset(xb3[:, 1:65, 65], 0.0)
            for half in range(2):
                nc.sync.dma_start(
                    x_sb[ct][:, half * (N // 2):(half + 1) * (N // 2)],
                    x_ap[ct].rearrange("p h w -> p (h w)")[:, half * (N // 2):
                                                          (half + 1) * (N // 2)])
            nc.sync.dma_start(qks_sb[ct][:], qks_ap[ct])
            nc.sync.dma_start(qkt_sb[ct][:], qkt_ap[ct])
            for t in range(9):
                nc.sync.dma_start(
                    pegd_sb[:, (ct * 9 + t) * P:(ct * 9 + t + 1) * P],
                    pegd_ap[ct, t])
                nc.sync.dma_start(
                    qkd_sb[:, (ct * 9 + t) * P:(ct * 9 + t + 1) * P],
                    qkd_ap[ct, t])
        def load_proj_weights():
            for ct in range(2):
                nc.sync.dma_start(pbs_sb[ct][:], pbs_ap[ct])
                nc.sync.dma_start(pbt_sb[ct][:], pbt_ap[ct])
                nc.sync.dma_start(p1t_sb[:, ct * C:(ct + 1) * C], p1t_ap[ct])
                nc.sync.dma_start(p2t_sb[:, ct * HID:(ct + 1) * HID], p2t_ap[ct])
            for kt in range(8):
                nc.sync.dma_start(p3t_sb[:, kt * C:(kt + 1) * C], p3t_ap[kt])

        # constants / pools used by the compute body
        att_pool = ctx.enter_context(tc.tile_pool(name="att", bufs=3))
        epi_pool = ctx.enter_context(tc.tile_pool(name="epi", bufs=4))
        proj_pool = ctx.enter_context(tc.tile_pool(name="proj", bufs=4))
        out_pool = ctx.enter_context(tc.tile_pool(name="outp", bufs=4))
        ident = pers.tile([P, P], BF16, name="ident")
        make_identity(nc, ident)
        nc.gpsimd.memset(vT3[:, :, 256:257], 1.0)

        def emit_body():
            # bf16 padded input (conv rhs); borders stay zero from the memset
            for ct in range(2):
                for half in range(2):
                    nc.vector.tensor_copy(
                        xp3b[ct][:, 1 + half * 32:1 + (half + 1) * 32, 1:65],
                        x_sb[ct].rearrange("p (h w) -> p h w", h=H, w=W)
                        [:, half * 32:(half + 1) * 32, :])

            # ---- PEG conv (V branch) on PE ----
            for ct in range(2):
                def evict_v(nb, vp, ct=ct):
                    nc.scalar.copy(v_sb[ct][:, nb * NT:(nb + 1) * NT], vp[:])
                _conv_pe(nc, pmm, pegd_sb, ct * 9, xp3b[ct], evict_v)

            # ---- QK conv interleaved with vT transposes (PE works while DVE
            # evicts the transposed tiles) ----
            def emit_vt_group(g):
                # 8 transposes per group g in 0..7
                for i in range(8):
                    mi, ct = divmod(g * 8 + i, 2)
                    vtp = pzt.tile([P, P], BF16, name="vtp",
                                   tag=f"zt{(g * 8 + i) % 4}", bufs=1)
                    nc.tensor.transpose(
                        vtp[:], v_sb[ct][:, mi * P:(mi + 1) * P], ident[:])
                    nc.vector.tensor_copy(
                        vT3[:, mi, ct * P:(ct + 1) * P], vtp[:])

            g = 0
            for ct in range(2):
                def evict_qk(nb, vp, ct=ct):
                    nc.scalar.activation(
                        qk_sb[ct][:, nb * NT:(nb + 1) * NT], vp[:], AF.Silu,
                        bias=qkt_sb[ct][:], scale=qks_sb[ct][:])
                for nb in range(NB):
                    if nb % 2 == 0:
                        emit_vt_group(g)
                        g += 1
                    vp = pmm.tile([P, NT], F32, name="convp", tag="mm")
                    ti = 0
                    for dy in range(3):
                        for dx in range(3):
                            win = xp3b[ct][:, dy + nb * 8:dy + nb * 8 + 8,
                                           dx:dx + W]
                            nc.tensor.matmul(
                                vp[:], qkd_sb[:, (ct * 9 + ti) * P:
                                              (ct * 9 + ti + 1) * P],
                                win, start=(ti == 0), stop=(ti == 8))
                            ti += 1
                    evict_qk(nb, vp)

            # ---- attention (software-pipelined: lg of pair i+1 before zt of i) ----
            load_proj_weights()
            q, k = qk_sb[0], qk_sb[1]
            NPAIR = MT // 2

            def emit_lg_pair(nt, mp):
                tiles = []
                for h in range(2):
                    mi = 2 * mp + h
                    lg = pmm.tile([P, NT], F32, name="lg", tag="mm")
                    nc.tensor.matmul(
                        lg[:], k[:, mi * P:(mi + 1) * P],
                        q[:, nt * NT:(nt + 1) * NT], start=True, stop=True)
                    tiles.append(lg)
                return tiles

            def pair_seq():
                for nt in range(NB):
                    for mp in range(NPAIR):
                        yield nt, mp

            seq = list(pair_seq())
            pend = {}
            pend[seq[0]] = emit_lg_pair(*seq[0])
            pend[seq[1]] = emit_lg_pair(*seq[1])
            for idx, (nt, mp) in enumerate(seq):
                if mp == 0:
                    zt = [pzt.tile([P, 257], F32, name=f"ztp{j}", tag=f"zt{j}",
                                   bufs=1) for j in range(4)]
                lg2 = pend.pop((nt, mp))
                et = att_pool.tile([P, 2 * NT], FP8, name="et", tag="et", bufs=3)
                # two halves of the pair exp'd concurrently on ACT and DVE
                for h in range(2):
                    dve = mp >= 3 and h == mp % 2
                    if dve:
                        nc.vector._custom_dve(
                            _EXP_OP, out=et[:, h * NT:(h + 1) * NT],
                            in0=lg2[h][:],
                            s0=EXP_C[0], s1=EXP_C[1], imm2=EXP_C[2])
                    else:
                        nc.scalar.activation(et[:, h * NT:(h + 1) * NT],
                                             lg2[h][:], AF.Exp, scale=1.0 / 16.0)
                if idx + 2 < len(seq):
                    pend[seq[idx + 2]] = emit_lg_pair(*seq[idx + 2])
                et3 = et.rearrange("p (h n) -> p h n", h=2, n=NT)
                for j in range(4):
                    nc.tensor.matmul(
                        zt[j][:],
                        et3[:, :, j * P:(j + 1) * P],
                        vT3[:, 2 * mp:2 * mp + 2, :],
                        start=(mp == 0), stop=(mp == NPAIR - 1),
                        perf_mode=mybir.MatmulPerfMode.DoubleRow)
                if mp == NPAIR - 1:
                    for jj in range(2):  # j pairs (2j, 2j+1): shared evictions
                        tps = []
                        for dj in range(2):
                            j = 2 * jj + dj
                            recip = epi_pool.tile([P, 1], F32, name="recip",
                                                  tag="recip")
                            nc.vector.reciprocal(recip[:], zt[j][:, 256:257])
                            zn = epi_pool.tile([P, C], BF16, name="zn", tag="zn")
                            nc.vector.tensor_scalar(zn[:], zt[j][:, :C],
                                                    recip[:], None, ALU.mult)
                            for ct in range(2):
                                tp2 = pmm.tile([P, 2 * P], BF16, name="tp2",
                                               tag="mm")
                                if dj == 0:
                                    tps.append(tp2)
                                else:
                                    tp2 = tps[ct]
                                nc.tensor.transpose(
                                    tp2[:, dj * P:(dj + 1) * P],
                                    zn[:, ct * P:(ct + 1) * P], ident[:])
                        for ct in range(2):
                            nc.vector.tensor_copy(
                                z_sb[ct][:, nt * NT + 2 * jj * P:
                                          nt * NT + (2 * jj + 2) * P],
                                tps[ct][:])

            if dbg:
                nc.sync.dma_start(dbg_aps["dq"][:], qk_sb[0][:])
                nc.sync.dma_start(dbg_aps["dk"][:], qk_sb[1][:])
                nc.sync.dma_start(dbg_aps["dz0"][:], z_sb[0][:])
                nc.sync.dma_start(dbg_aps["dz1"][:], z_sb[1][:])
                nc.sync.dma_start(dbg_aps["dvT"][:], vT[:])

            # ---- projection MLP: p1 -> BN -> p2 -> SiLU -> p3(*gamma) + x ----
            for nt in range(NB):
                ns = slice(nt * NT, (nt + 1) * NT)
                h1pair = proj_pool.tile([P, 2 * NT], FP8, name="h1pair",
                                        tag="h1", bufs=3)
                for ot in range(2):
                    h1p = pzt.tile([P, NT], F32, name="h1p", tag=f"zt{ot}", bufs=1)
                    for kt in range(2):
                        nc.tensor.matmul(
                            h1p[:],
                            p1t_sb[:, kt * C + ot * P:kt * C + (ot + 1) * P],
                            z_sb[kt][:, ns], start=(kt == 0), stop=(kt == 1))
                    nc.vector.tensor_scalar(h1pair[:, ot * NT:(ot + 1) * NT],
                                            h1p[:], pbs_sb[ot][:],
                                            pbt_sb[ot][:], ALU.mult, ALU.add)
                h1pr = h1pair.rearrange("p (a b) -> p a b", a=2, b=NT)
                p2pr = p2t_sb.rearrange("p (a b) -> p a b", a=2, b=HID)
                h2 = []
                for g in range(4):
                    h2t = proj_pool.tile([P, 2 * NT], FP8, name="h2t", tag="h2",
                                         bufs=6)
                    h2.append(h2t)
                for ht in range(8):
                    h2p = pmm.tile([P, NT], F32, name="h2p", tag="mm")
                    nc.tensor.matmul(
                        h2p[:], p2pr[:, :, ht * P:(ht + 1) * P], h1pr[:],
                        start=True, stop=True,
                        perf_mode=mybir.MatmulPerfMode.DoubleRow)
                    nc.scalar.activation(
                        h2[ht // 2][:, (ht % 2) * NT:(ht % 2 + 1) * NT],
                        h2p[:], AF.Silu)
                for ot in range(2):
                    zfp = pzt.tile([P, NT], F32, name="zfp", tag=f"zt{ot + 2}",
                                   bufs=1)
                    for g in range(4):
                        p3pr = p3t_sb[:, 2 * g * C:(2 * g + 2) * C].rearrange(
                            "p (a b) -> p a b", a=2, b=C)
                        h2pr = h2[g].rearrange("p (a b) -> p a b", a=2, b=NT)
                        nc.tensor.matmul(
                            zfp[:], p3pr[:, :, ot * P:(ot + 1) * P], h2pr[:],
                            start=(g == 0), stop=(g == 3),
                            perf_mode=mybir.MatmulPerfMode.DoubleRow)
                    ob = out_pool.tile([P, NT], F32, name="ob", tag="ob")
                    nc.vector.tensor_tensor(ob[:], zfp[:], x_sb[ot][:, ns], ALU.add)
                    nc.sync.dma_start(out_ap[ot][:, ns], ob[:])

        for _rep in range(reps):
            emit_body()

    nc.finalize()
    return nc


def _prep_inputs(x, qk_w, qk_g, qk_b, qk_m, qk_v, peg_w,
                 p1_w, pb_g, pb_b, pb_m, pb_v, p2_w, p3_w, gamma):
    f32 = np.float32
    bf16 = ml_dtypes.bfloat16
    fp8 = ml_dtypes.float8_e4m3

    def diag_taps(w9):
        d = np.zeros((2, 9, P, P), f32)
        idx = np.arange(P)
        for ct in range(2):
            for t in range(9):
                d[ct, t, idx, idx] = w9[ct * P:(ct + 1) * P, t]
        return d

    qks = (qk_g / np.sqrt(qk_v + EPS)).astype(f32)
    qkt = (qk_b - qk_m * qks).astype(f32)
    qkd = diag_taps(np.asarray(qk_w, f32).reshape(C, 9))

    pegw = np.asarray(peg_w, f32).reshape(C, 9).copy()
    pegw[:, 4] += 1.0  # fold +x residual into center tap
    pegd = diag_taps(pegw)

    pbs = (pb_g / np.sqrt(pb_v + EPS)).astype(f32)
    pbt = (pb_b - pb_m * pbs).astype(f32)

    p1t = np.ascontiguousarray(np.asarray(p1_w, f32).T).reshape(2, P, C)
    p2t = np.ascontiguousarray(np.asarray(p2_w, f32).T).reshape(2, P, HID)
    p3g = np.asarray(p3_w, f32) * np.asarray(gamma, f32)[:, None]
    p3t = np.ascontiguousarray(p3g.T).reshape(8, P, C)

    shared = {
        "qkd": qkd.astype(bf16),
        "qks": qks.reshape(2, P, 1).astype(f32),
        "qkt": qkt.reshape(2, P, 1).astype(f32),
        "pegd": pegd.astype(bf16),
        "p1t": p1t.astype(bf16),
        "pbs": pbs.reshape(2, P, 1).astype(f32),
        "pbt": pbt.reshape(2, P, 1).astype(f32),
        "p2t": p2t.astype(fp8),
        "p3t": p3t.astype(fp8),
    }
    xs = np.asarray(x, f32).reshape(8, 2, P, H, W)
    return [dict(shared, x=np.ascontiguousarray(xs[i])) for i in range(N_CORES)]


def kernel(**inputs):
    if "nc" not in _cache:
        _cache["nc"] = _build_program()
    nc = _cache["nc"]
    in_maps = _prep_inputs(**inputs)
    res = run_bass_kernel_spmd(nc, in_maps, list(range(N_CORES)))
    _cache["last_result"] = res
    out = np.stack([res.results[i]["out"].reshape(C, H, W)
                    for i in range(N_CORES)])
    return out.astype(np.float32)
